# revision 1
# baseline (speedup 1.0000x reference)
"""Trainium2 Bass kernel for the AugmentedNeuralODE problem.

Data-parallel over 8 NeuronCores: each core integrates 256 samples of the
batch through 255 RK4 steps of the decoder-MLP vector field.

On-chip layout (per core):
  state zT:   [D=128 partitions, BS=256]   (features on partitions)
  weights:    W1 [128, 512], W2 [128, 4, 512], W3 [128, 4, 128]  (K-major,
              so 128x128 blocks are direct matmul lhsT operands)
  matmul operands in float32r (rounded fp32) or fp16/bf16

Self-contained: hardcodes shapes/sharding; no sibling imports.
"""

import numpy as np

B, T = 2048, 256
D_DATA, AUG, H = 125, 3, 512
D = 128
NCORES = 8
BS = B // NCORES          # 256 samples per core
NS = T - 1                # 255 integration steps
CHUNK = 15                # steps per hardware-loop iteration (255 = 17*15)

_CACHE = {}


def _build(reps=1, n_steps=NS, chunk=CHUNK, dt_mode="gp", mm="fp16",
           halves=True, timing_mode=False, wide_act=True, fast=None):
    import concourse.bass as bass
    import concourse.tile as tile
    from concourse import bacc, mybir

    dt = mybir.dt
    f32 = dt.float32

    nc = bacc.Bacc("TRN2", target_bir_lowering=False, debug=False)

    z0_d = nc.dram_tensor("z0T", [D, BS], f32, kind="ExternalInput").ap()
    dt_d = nc.dram_tensor("dtT", [max(n_steps, 1), BS], f32,
                          kind="ExternalInput").ap()
    w1_d = nc.dram_tensor("w1x", [128, H], f32, kind="ExternalInput").ap()
    w2_d = nc.dram_tensor("w2x", [128, 4, H], f32, kind="ExternalInput").ap()
    w3_d = nc.dram_tensor("w3x", [128, 4, D], f32, kind="ExternalInput").ap()
    b1_d = nc.dram_tensor("b1x", [128, 4], f32, kind="ExternalInput").ap()
    b2_d = nc.dram_tensor("b2x", [128, 4], f32, kind="ExternalInput").ap()
    b3_d = nc.dram_tensor("b3x", [128, 1], f32, kind="ExternalInput").ap()
    if timing_mode:
        # same per-step DMA traffic, but to an internal scratch buffer so
        # nothing big crosses the host link; tiny real output for liveness
        out_d = nc.dram_tensor("zs_scratch", [max(n_steps, 1), D, BS],
                               f32).ap()
        fin_d = nc.dram_tensor("zfin", [D, BS], f32,
                               kind="ExternalOutput").ap()
    else:
        out_d = nc.dram_tensor("zs", [max(n_steps, 1), D, BS], f32,
                               kind="ExternalOutput").ap()
        fin_d = None

    with tile.TileContext(nc) as tc:
        _emit(tc, bass, mybir, z0_d, dt_d, w1_d, w2_d, w3_d, b1_d, b2_d,
              b3_d, out_d, reps, n_steps, chunk, dt_mode, mm, halves,
              fin_d, wide_act, fast)
    nc.compile()
    return nc


def _emit(tc, bass, mybir, z0_d, dt_d, w1_d, w2_d, w3_d, b1_d, b2_d, b3_d,
          out_d, reps, n_steps, chunk, dt_mode, mm, halves, fin_d=None,
          wide_act=True, fast=None):
    from contextlib import ExitStack

    dt = mybir.dt
    f32 = dt.float32
    mmdt = {"f32r": dt.float32r, "bf16": dt.bfloat16,
            "fp16": dt.float16}[mm]
    AF = mybir.ActivationFunctionType
    Alu = mybir.AluOpType
    ds = bass.ds
    ts = bass.ts
    nc = tc.nc

    def rd(ap):
        """Readable view of an mm-dtype AP for DVE (f32r bits are f32)."""
        return ap.bitcast(f32) if mm == "f32r" else ap

    ctx = ExitStack()
    with ctx:
        wp = ctx.enter_context(tc.tile_pool(name="wp", bufs=1))
        setup = ctx.enter_context(tc.tile_pool(name="setup", bufs=2))
        sb = ctx.enter_context(tc.tile_pool(name="sb", bufs=2))
        # deeper h-tile rotation for cross-stage scheduling freedom
        dtp = ctx.enter_context(tc.tile_pool(name="dtp", bufs=3))
        hp = ctx.enter_context(tc.tile_pool(name="hp", bufs=2))
        psw = ctx.enter_context(tc.tile_pool(name="psw", bufs=1, space="PSUM"))
        psk = ctx.enter_context(tc.tile_pool(name="psk", bufs=3, space="PSUM"))

        # ---- weights: DMA f32 then round once to mm dtype ----
        w1r = wp.tile([128, H], mmdt)
        w2r = wp.tile([128, 4, H], mmdt)
        w3r = wp.tile([128, 4, D], mmdt)
        for dst, src in ((w1r, w1_d), (w2r, w2_d), (w3r, w3_d)):
            tmp = setup.tile(list(dst.shape), f32, tag="wtmp")
            nc.sync.dma_start(tmp[:], src[:])
            nc.vector.tensor_copy(dst[:], tmp[:])

        b1t = wp.tile([128, 4], f32)
        b2t = wp.tile([128, 4], f32)
        b3t = wp.tile([128, 1], f32)
        nc.sync.dma_start(b1t[:], b1_d[:])
        nc.sync.dma_start(b2t[:], b2_d[:])
        nc.sync.dma_start(b3t[:], b3_d[:])

        zA = wp.tile([D, BS], mmdt)
        zB = wp.tile([D, BS], mmdt)
        # f32 master state (only used for mm != f32r)
        zF = wp.tile([D, BS], f32)

        def init_state():
            z0t = setup.tile([D, BS], f32, tag="z0tmp")
            nc.sync.dma_start(z0t[:], z0_d[:])
            nc.vector.tensor_copy(zA[:], z0t[:])
            if mm != "f32r":
                nc.vector.tensor_copy(zF[:], z0t[:])

        # half-split column ranges within a BS-wide chunk
        if halves:
            hsplits = [(0, 128), (128, 128)]
        else:
            hsplits = [(0, BS)]

        def mlp12(z_r):
            """Layers 1+2 (+tanh): z_r [128,BS] mmdt -> h2 [128,4*BS] mmdt."""
            nh = len(hsplits)
            h1 = hp.tile([128, 4 * BS], mmdt, tag="h1")
            p1t = [psw.tile([128, 2 * BS], f32, tag=f"pw1{i}",
                            name=f"pw1{i}") for i in range(2)]
            for ti, pt in enumerate(p1t):
                n_mm = 2 * nh
                j = 0
                for m_loc in range(2):
                    m = ti * 2 + m_loc
                    for off, w in hsplits:
                        nc.tensor.matmul(
                            pt[:, m_loc * BS + off:m_loc * BS + off + w],
                            w1r[:, ts(m, 128)], z_r[:, off:off + w],
                            start=(j == 0), stop=(j == n_mm - 1))
                        j += 1
                if wide_act:
                    nc.scalar.activation(h1[:, ts(ti, 2 * BS)], pt[:],
                                         AF.Tanh)
                else:
                    for m_loc in range(2):
                        m = ti * 2 + m_loc
                        nc.scalar.activation(h1[:, ts(m, BS)],
                                             pt[:, ts(m_loc, BS)],
                                             AF.Tanh, bias=b1t[:, m:m + 1])
            h2 = hp.tile([128, 4 * BS], mmdt, tag="h2")
            p2t = [psw.tile([128, 2 * BS], f32, tag=f"pw2{i}",
                            name=f"pw2{i}") for i in range(2)]
            for ti, pt in enumerate(p2t):
                for k in range(4):
                    for m_loc in range(2):
                        m = ti * 2 + m_loc
                        for hi, (off, w) in enumerate(hsplits):
                            nc.tensor.matmul(
                                pt[:, m_loc * BS + off:m_loc * BS + off + w],
                                w2r[:, k, ts(m, 128)],
                                h1[:, k * BS + off:k * BS + off + w],
                                start=(k == 0 and m_loc == 0 and hi == 0),
                                stop=(k == 3 and m_loc == 1 and hi == nh - 1))
                if wide_act:
                    nc.scalar.activation(h2[:, ts(ti, 2 * BS)], pt[:],
                                         AF.Tanh)
                else:
                    for m_loc in range(2):
                        m = ti * 2 + m_loc
                        nc.scalar.activation(h2[:, ts(m, BS)],
                                             pt[:, ts(m_loc, BS)],
                                             AF.Tanh, bias=b2t[:, m:m + 1])
            return h2

        def l3_raw(h2, k_out):
            """k_out (PSUM) = W3.T @ h2 (fresh accumulation group)."""
            nh = len(hsplits)
            for k in range(4):
                for hi, (off, w) in enumerate(hsplits):
                    nc.tensor.matmul(k_out[:, off:off + w], w3r[:, k, :],
                                     h2[:, k * BS + off:k * BS + off + w],
                                     start=(k == 0 and hi == 0),
                                     stop=(k == 3 and hi == nh - 1))

        def l3_scaled_into(h2, scale_ap, zps):
            """zps (PSUM, preloaded with the base state) += W3.T@(scale*h2).

            scale is per-column; it commutes with W3.T so scaling h2 gives
            z_next = base + scale*(W3.T@h2) directly in PSUM. The preload
            is emitted by the caller BEFORE mlp12 so it never gates PE.
            """
            nh = len(hsplits)
            h2s = hp.tile([128, 4 * BS], mmdt, tag="h2s")
            for ch in range(4):
                nc.vector.tensor_mul(h2s[:, ts(ch, BS)], h2[:, ts(ch, BS)],
                                     scale_ap[:])
            for k in range(4):
                for hi, (off, w) in enumerate(hsplits):
                    nc.tensor.matmul(zps[:, off:off + w], w3r[:, k, :],
                                     h2s[:, k * BS + off:k * BS + off + w],
                                     start=False,
                                     stop=(k == 3 and hi == nh - 1),
                                     skip_group_check=True)

        def step_fast(t_sv, z_cur, z_nxt, dt_row):
            """RK4 step, zero-b1/b2 fast path (any b3 via q folding).

            Stage boundaries keep only one PSUM->SBUF copy on the critical
            path: z_{i+1} is accumulated in PSUM by layer 3 itself.
            z_new = (2/3)zb_f - zb_h + (z2 + 2*z3 + z4)/3 + W3.T@(dtb/6*h2_4)
            """
            z_f = zF
            dtb = sb.tile([128, BS], f32, tag="dtb")
            if dt_mode == "gp":
                nc.gpsimd.partition_broadcast(dtb[:], dt_row)
            else:
                nc.sync.dma_start(
                    dtb[:], dt_d[ds(t_sv, 1), :].to_broadcast((128, BS)))
            dtb_h = sb.tile([128, BS], f32, tag="dtbh")
            nc.vector.tensor_scalar_mul(dtb_h[:], dtb[:], 0.5)
            dtb_6 = sb.tile([128, BS], f32, tag="dtb6")
            nc.vector.tensor_scalar_mul(dtb_6[:], dtb[:], 1.0 / 6.0)
            q = sb.tile([128, BS], f32, tag="q")
            nc.vector.tensor_scalar(q[:], dtb[:], b3t[:, 0:1], None, Alu.mult)
            zb_h = sb.tile([128, BS], f32, tag="zbh")
            nc.vector.scalar_tensor_tensor(zb_h[:], q[:], 0.5, z_f[:],
                                           Alu.mult, Alu.add)
            zb_f = sb.tile([128, BS], f32, tag="zbf")
            nc.vector.tensor_add(zb_f[:], q[:], z_f[:])
            G = sb.tile([128, BS], f32, tag="G")
            nc.vector.tensor_scalar_mul(G[:], z_f[:], -1.0 / 3.0)
            nc.vector.scalar_tensor_tensor(G[:], q[:], 1.0 / 6.0, G[:],
                                           Alu.mult, Alu.add)

            # stage 1
            z2ps = psk.tile([128, BS], f32, tag="k")
            nc.vector.tensor_copy(z2ps[:], zb_h[:])
            h2 = mlp12(z_cur)
            l3_scaled_into(h2, dtb_h, z2ps)
            z2 = sb.tile([128, BS], mmdt, tag="z2")
            nc.vector.tensor_copy(z2[:], z2ps[:])
            z2f = sb.tile([128, BS], f32, tag="z2f")
            nc.vector.tensor_copy(z2f[:], z2ps[:])
            nc.vector.scalar_tensor_tensor(G[:], z2f[:], 1.0 / 3.0, G[:],
                                           Alu.mult, Alu.add)
            # stage 2
            z3ps = psk.tile([128, BS], f32, tag="k")
            nc.vector.tensor_copy(z3ps[:], zb_h[:])
            h2 = mlp12(z2)
            l3_scaled_into(h2, dtb_h, z3ps)
            z3 = sb.tile([128, BS], mmdt, tag="z3")
            nc.vector.tensor_copy(z3[:], z3ps[:])
            z3f = sb.tile([128, BS], f32, tag="z3f")
            nc.vector.tensor_copy(z3f[:], z3ps[:])
            nc.vector.scalar_tensor_tensor(G[:], z3f[:], 2.0 / 3.0, G[:],
                                           Alu.mult, Alu.add)
            # stage 3
            z4ps = psk.tile([128, BS], f32, tag="k")
            nc.vector.tensor_copy(z4ps[:], zb_f[:])
            h2 = mlp12(z3)
            l3_scaled_into(h2, dtb, z4ps)
            z4 = sb.tile([128, BS], mmdt, tag="z4")
            nc.vector.tensor_copy(z4[:], z4ps[:])
            z4f = sb.tile([128, BS], f32, tag="z4f")
            nc.vector.tensor_copy(z4f[:], z4ps[:])
            nc.vector.scalar_tensor_tensor(G[:], z4f[:], 1.0 / 3.0, G[:],
                                           Alu.mult, Alu.add)
            # stage 4
            znps = psk.tile([128, BS], f32, tag="k")
            nc.vector.tensor_copy(znps[:], G[:])
            h2 = mlp12(z4)
            l3_scaled_into(h2, dtb_6, znps)
            nc.vector.tensor_copy(z_nxt[:], znps[:])
            nc.vector.tensor_copy(zF[:], znps[:])
            nc.sync.dma_start(out_d[ds(t_sv, 1), :, :], zF[:])

        def step_generic(t_sv, z_cur, z_nxt, dt_row):
            """One RK4 step, generic-bias path."""
            z_f = rd(z_cur) if mm == "f32r" else zF
            dtb = sb.tile([128, BS], f32, tag="dtb")
            if dt_mode == "gp":
                nc.gpsimd.partition_broadcast(dtb[:], dt_row)
            else:
                nc.sync.dma_start(
                    dtb[:], dt_d[ds(t_sv, 1), :].to_broadcast((128, BS)))
            dtb_h = sb.tile([128, BS], f32, tag="dtbh")
            nc.gpsimd.tensor_scalar_mul(dtb_h[:], dtb[:], 0.5)
            dtb_6 = sb.tile([128, BS], f32, tag="dtb6")
            nc.gpsimd.tensor_scalar_mul(dtb_6[:], dtb[:], 1.0 / 6.0)
            q = sb.tile([128, BS], f32, tag="q")
            nc.vector.tensor_scalar(q[:], dtb[:], b3t[:, 0:1], None, Alu.mult)
            zb_h = sb.tile([128, BS], f32, tag="zbh")
            nc.vector.scalar_tensor_tensor(zb_h[:], q[:], 0.5, z_f[:],
                                           Alu.mult, Alu.add)
            zb_f = sb.tile([128, BS], f32, tag="zbf")
            nc.gpsimd.tensor_add(zb_f[:], q[:], z_f[:])

            k1 = psk.tile([128, BS], f32, tag="k")
            h2 = mlp12(z_cur)
            l3_raw(h2, k1)
            tmp = sb.tile([128, BS], f32, tag="tmp")
            nc.vector.tensor_mul(tmp[:], dtb_h[:], k1[:])
            z2 = sb.tile([128, BS], mmdt, tag="z2")
            nc.vector.tensor_add(z2[:], tmp[:], zb_h[:])

            k2 = psk.tile([128, BS], f32, tag="k")
            h2 = mlp12(z2)
            l3_raw(h2, k2)
            tmp2 = sb.tile([128, BS], f32, tag="tmp")
            nc.vector.tensor_mul(tmp2[:], dtb_h[:], k2[:])
            z3 = sb.tile([128, BS], mmdt, tag="z3")
            nc.vector.tensor_add(z3[:], tmp2[:], zb_h[:])
            s = sb.tile([128, BS], f32, tag="s")
            nc.vector.tensor_scalar_mul(s[:], k2[:], 2.0)
            nc.vector.tensor_add(s[:], s[:], k1[:])

            k3 = psk.tile([128, BS], f32, tag="k")
            h2 = mlp12(z3)
            l3_raw(h2, k3)
            tmp3 = sb.tile([128, BS], f32, tag="tmp")
            nc.vector.tensor_mul(tmp3[:], dtb[:], k3[:])
            z4 = sb.tile([128, BS], mmdt, tag="z4")
            nc.vector.tensor_add(z4[:], tmp3[:], zb_f[:])
            nc.vector.scalar_tensor_tensor(s[:], k3[:], 2.0, s[:],
                                           Alu.mult, Alu.add)

            k4 = psk.tile([128, BS], f32, tag="k")
            h2 = mlp12(z4)
            l3_raw(h2, k4)
            t1 = sb.tile([128, BS], f32, tag="t1")
            nc.vector.tensor_add(t1[:], s[:], k4[:])
            t2 = sb.tile([128, BS], f32, tag="t2")
            nc.vector.tensor_mul(t2[:], t1[:], dtb_6[:])
            nc.vector.tensor_add(z_nxt[:], t2[:], zb_f[:])
            if mm == "f32r":
                out_src = z_nxt.bitcast(f32)[:]
            else:
                nc.vector.tensor_add(zF[:], t2[:], zb_f[:])
                out_src = zF[:]
            nc.sync.dma_start(out_d[ds(t_sv, 1), :, :], out_src)

        def run_chunk(iv, nsteps_c):
            """Emit nsteps_c steps; iv is loop var (or int when static)."""
            dtc = dtp.tile([1, max(nsteps_c, 1), BS], f32, tag="dtc")
            nc.sync.dma_start(dtc[:], dt_d[ds(iv, nsteps_c), :][None])
            use_fast = False if fast is None else fast
            step = step_fast if use_fast else step_generic
            for u in range(nsteps_c):
                zc, zn = (zA, zB) if u % 2 == 0 else (zB, zA)
                step(iv + u, zc, zn, dtc[0:1, u, :])
            if nsteps_c % 2 == 1:
                nc.vector.tensor_copy(zA[:], rd(zB)[:])

        def run_integration():
            init_state()
            n_loop = 0 if chunk == 0 else (n_steps // chunk) * chunk
            if n_loop:
                with tc.For_i(0, n_loop, chunk,
                              hint_engines=(mybir.EngineType.PE,)) as iv:
                    run_chunk(iv, chunk)
            if n_loop < n_steps:
                run_chunk(n_loop, n_steps - n_loop)

        if reps > 1:
            with tc.For_i(0, reps, 1):
                run_integration()
        else:
            run_integration()
        if fin_d is not None:
            zlast = zA if mm == "f32r" else zF
            nc.sync.dma_start(fin_d[:], rd(zlast)[:])


def _prep_in_maps(x_ivps, t_seg, seg_lens, W1, b1, W2, b2, W3, b3):
    x_ivps = np.asarray(x_ivps, np.float32)
    t_seg = np.asarray(t_seg, np.float32)
    seg_lens = np.asarray(seg_lens)
    dt_raw = t_seg[:, 1:] - t_seg[:, :-1]                    # [B, NS]
    step_idx = np.arange(1, T)
    valid = step_idx[None, :] < seg_lens[:, None]
    dtm = np.where(valid, dt_raw, 0.0).astype(np.float32)    # [B, NS]

    z0 = np.concatenate(
        [x_ivps[:, 0, :], np.zeros((B, AUG), np.float32)], axis=1)  # [B, D]

    W1 = np.asarray(W1, np.float32)
    W2 = np.asarray(W2, np.float32)
    W3 = np.asarray(W3, np.float32)
    w2x = np.ascontiguousarray(W2.reshape(4, 128, H).transpose(1, 0, 2))
    w3x = np.ascontiguousarray(W3.reshape(4, 128, D).transpose(1, 0, 2))
    b1x = np.ascontiguousarray(np.asarray(b1, np.float32).reshape(4, 128).T)
    b2x = np.ascontiguousarray(np.asarray(b2, np.float32).reshape(4, 128).T)
    b3x = np.ascontiguousarray(np.asarray(b3, np.float32).reshape(D, 1))

    in_maps = []
    for c in range(NCORES):
        sl = slice(c * BS, (c + 1) * BS)
        in_maps.append({
            "z0T": np.ascontiguousarray(z0[sl].T),
            "dtT": np.ascontiguousarray(dtm[sl].T),
            "w1x": W1, "w2x": w2x, "w3x": w3x,
            "b1x": b1x, "b2x": b2x, "b3x": b3x,
        })
    return in_maps, z0


def kernel(x_ivps, t_seg, seg_lens, W1, b1, W2, b2, W3, b3):
    from concourse import bass_utils

    wide = bool(np.all(np.asarray(b1) == 0) and np.all(np.asarray(b2) == 0))
    key = ("nc", wide)
    if key not in _CACHE:
        _CACHE[key] = _build(wide_act=wide)
    nc = _CACHE[key]

    in_maps, z0 = _prep_in_maps(x_ivps, t_seg, seg_lens, W1, b1, W2, b2,
                                W3, b3)
    res = bass_utils.run_bass_kernel_spmd(
        nc, in_maps, core_ids=list(range(NCORES)))

    sol = np.empty((B, T, 1, D), np.float32)
    sol[:, 0, 0, :] = z0
    for c in range(NCORES):
        zs = res.results[c]["zs"]                  # [NS, D, BS]
        sol[c * BS:(c + 1) * BS, 1:, 0, :] = zs.transpose(2, 0, 1)
    return sol

